# revision 4
# baseline (speedup 1.0000x reference)
"""AdaptiveTripletLoss distributed Trainium2 kernel (8 NeuronCores).

Strategy: shard by class. Host argsorts targets; each class becomes one
128-row padded block (max class count is ~105 for n=8192, C=100). 104
class slots = 13 blocks/core x 8 cores. Hardest-positive top-3 needs only
same-class distances, so each core computes 13 small 128x128 gram blocks
instead of a row-slab of the full 8192x8192 matrix. Class centers are
computed per-class locally and AllGathered. All floating-point loss math
runs on device; the host does data movement (sharding permutation) and the
final 8-way partial sum / count division.
"""

import numpy as np
from concourse import bacc, mybir, tile, masks
from concourse.bass_utils import run_bass_kernel_spmd

# Problem constants (hardcoded per harness contract)
N = 8192
D = 512
C = 100
NCORES = 8
BPC = 13            # class blocks per core
NSLOT = BPC * NCORES  # 104 class slots
P = 128             # rows per class block
KCH = D // P        # 4 contraction chunks
BIG = 1.0e4
EPS = 1.0e-12
F32 = mybir.dt.float32

_CACHED_NC = None


def _build_nc():
    nc = bacc.Bacc("TRN2", target_bir_lowering=False, debug=False,
                   num_devices=NCORES)
    emb_h = nc.declare_dram_parameter("emb", [BPC * P, D], F32, isOutput=False)
    rwm_h = nc.declare_dram_parameter("rwm", [P, BPC * BPC], F32, isOutput=False)
    lw_h = nc.declare_dram_parameter("lw", [P, BPC], F32, isOutput=False)
    pb_h = nc.declare_dram_parameter("padbias", [1, BPC * P], F32, isOutput=False)
    nb_h = nc.declare_dram_parameter("negbias", [1, BPC * NSLOT], F32, isOutput=False)
    out_h = nc.declare_dram_parameter("out", [1, 1], F32, isOutput=True)

    AX = mybir.AxisListType
    OP = mybir.AluOpType
    AF = mybir.ActivationFunctionType

    with tile.TileContext(nc) as tc:
        with (
            tc.tile_pool(name="const", bufs=1) as cpool,
            tc.tile_pool(name="big", bufs=1) as bpool,
            tc.tile_pool(name="sm", bufs=1) as spool,
            tc.tile_pool(name="scr", bufs=3) as scr,
            tc.tile_pool(name="gt", bufs=3) as gt,
            tc.tile_pool(name="st", bufs=3) as st,
            tc.tile_pool(name="ps_t", bufs=3, space="PSUM") as ps_t,
            tc.tile_pool(name="ps_a", bufs=3, space="PSUM") as ps_a,
            tc.tile_pool(name="ps_c", bufs=1, space="PSUM") as ps_c,
            tc.tile_pool(name="dram", bufs=1, space="DRAM") as dram,
        ):
            # ---- constants ----
            ident = cpool.tile([P, P], F32, tag="ident")
            masks.make_identity(nc, ident[:])
            iota_i = cpool.tile([P, P], mybir.dt.int32, tag="iota_i")
            nc.gpsimd.iota(iota_i[:], pattern=[[1, P]], base=0, channel_multiplier=0)
            iota_f = cpool.tile([P, P], F32, tag="iota_f")
            nc.vector.tensor_copy(iota_f[:], iota_i[:])
            ones = cpool.tile([1, P], F32, tag="ones")
            nc.vector.memset(ones[:], 1.0)

            # ---- persistent tiles ----
            Eraw = bpool.tile([P, BPC * D], F32, tag="Eraw")
            E = bpool.tile([P, BPC * D], F32, tag="E")
            ET = bpool.tile([P, BPC * D], F32, tag="ET")
            rw_t = spool.tile([P, BPC * BPC], F32, tag="rw")
            lw_t = spool.tile([P, BPC], F32, tag="lwt")
            pb_t = spool.tile([1, BPC * P], F32, tag="pbt")
            nb_t = spool.tile([1, BPC * NSLOT], F32, tag="nbt")
            ssq = spool.tile([P, BPC], F32, tag="ssq")
            nrm = spool.tile([P, BPC], F32, tag="nrm")
            rcp = spool.tile([P, BPC], F32, tag="rcp")
            tsc = spool.tile([P, BPC], F32, tag="tsc")
            a2 = spool.tile([P, BPC], F32, tag="a2")
            dsqs = spool.tile([P, 2 * BPC], F32, tag="dsqs")  # cols 0..12 pos, 13..25 neg
            dpq = spool.tile([P, BPC], F32, tag="dpq")
            msc = spool.tile([P, BPC], F32, tag="msc")
            dnq = spool.tile([P, BPC], F32, tag="dnq")
            centers_l = spool.tile([BPC, D], F32, tag="centers_l")
            centers_all = spool.tile([NSLOT, D], F32, tag="centers_all")
            csq = spool.tile([NSLOT, D], F32, tag="csq")
            b2col = spool.tile([NSLOT, 1], F32, tag="b2col")
            comb = spool.tile([1, BPC * NSLOT], F32, tag="comb")
            ct2 = spool.tile([P, KCH * NSLOT], F32, tag="ct2")

            # ---- input DMAs ----
            for b in range(BPC):
                nc.sync.dma_start(out=Eraw[:, b * D:(b + 1) * D],
                                  in_=emb_h[b * P:(b + 1) * P, :])
            nc.sync.dma_start(out=rw_t[:], in_=rwm_h[:])
            nc.sync.dma_start(out=lw_t[:], in_=lw_h[:])
            nc.sync.dma_start(out=pb_t[:], in_=pb_h[:])
            nc.sync.dma_start(out=nb_t[:], in_=nb_h[:])

            # ---- phase 1: normalize rows ----
            for b in range(BPC):
                sq = scr.tile([P, D], F32, tag="sq")
                nc.scalar.activation(sq[:], Eraw[:, b * D:(b + 1) * D], AF.Square,
                                     accum_out=ssq[:, b:b + 1])
                nc.scalar.activation(nrm[:, b:b + 1], ssq[:, b:b + 1], AF.Sqrt)
                nc.vector.tensor_scalar_max(nrm[:, b:b + 1], nrm[:, b:b + 1], EPS)
                nc.vector.reciprocal(rcp[:, b:b + 1], nrm[:, b:b + 1])
                nc.scalar.activation(E[:, b * D:(b + 1) * D],
                                     Eraw[:, b * D:(b + 1) * D], AF.Copy,
                                     scale=rcp[:, b:b + 1])
                # a2 = ssq * rcp^2  (|normalized row|^2)
                nc.vector.tensor_mul(tsc[:, b:b + 1], rcp[:, b:b + 1], rcp[:, b:b + 1])
                nc.vector.tensor_mul(a2[:, b:b + 1], ssq[:, b:b + 1], tsc[:, b:b + 1])

            # ---- phase 2: per-class partial centers + AllGather ----
            pcn = ps_c.tile([BPC, D], F32, tag="pcn")
            for b in range(BPC):
                nc.tensor.matmul(pcn[:], lhsT=rw_t[:, b * BPC:(b + 1) * BPC],
                                 rhs=E[:, b * D:(b + 1) * D],
                                 start=(b == 0), stop=(b == BPC - 1))
            nc.vector.tensor_copy(centers_l[:], pcn[:])
            cc_in = dram.tile([BPC, D], F32, tag="cc_in")
            cc_out = dram.tile([NSLOT, D], F32, addr_space="Shared", tag="cc_out")
            nc.sync.dma_start(out=cc_in[:], in_=centers_l[:])
            nc.gpsimd.collective_compute(
                "AllGather", OP.bypass,
                replica_groups=[list(range(NCORES))],
                ins=[cc_in[:].opt()],
                outs=[cc_out[:].opt()],
            )
            nc.sync.dma_start(out=centers_all[:], in_=cc_out[:])

            # ---- phase 3: transpose E blocks ----
            for b in range(BPC):
                for k in range(KCH):
                    pt = ps_t.tile([P, P], F32, tag="pt")
                    nc.tensor.transpose(pt[:], E[:, b * D + k * P:b * D + (k + 1) * P],
                                        ident[:])
                    nc.vector.tensor_copy(ET[:, b * D + k * P:b * D + (k + 1) * P], pt[:])

            # ---- phase 4: gram + top3 + pos-center + d_pos^2 ----
            for b in range(BPC):
                pg = ps_a.tile([P, P], F32, tag="pa")
                for k in range(KCH):
                    sl = slice(b * D + k * P, b * D + (k + 1) * P)
                    nc.tensor.matmul(pg[:], lhsT=ET[:, sl], rhs=ET[:, sl],
                                     start=(k == 0), stop=False)
                nc.tensor.matmul(pg[:], lhsT=ones[:],
                                 rhs=pb_t[0:1, b * P:(b + 1) * P],
                                 start=False, stop=True)
                negG = gt.tile([P, P], F32, tag="negG")
                nc.vector.tensor_scalar_mul(negG[:], pg[:], -1.0)
                v8 = st.tile([P, 8], F32, tag="v8")
                nc.vector.max(v8[:], negG[:])
                i8 = st.tile([P, 8], mybir.dt.uint32, tag="i8")
                nc.vector.max_index(i8[:], v8[:], negG[:])
                idxf = st.tile([P, 3], F32, tag="idxf")
                nc.vector.tensor_copy(idxf[:], i8[:, 0:3])
                e0 = gt.tile([P, P], F32, tag="e0")
                nc.vector.tensor_scalar(e0[:], iota_f[:], idxf[:, 0:1], None,
                                        op0=OP.is_equal)
                e1 = gt.tile([P, P], F32, tag="e1")
                nc.vector.scalar_tensor_tensor(e1[:], in0=iota_f[:],
                                               scalar=idxf[:, 1:2], in1=e0[:],
                                               op0=OP.is_equal, op1=OP.add)
                S = gt.tile([P, P], F32, tag="S")
                nc.vector.scalar_tensor_tensor(S[:], in0=iota_f[:],
                                               scalar=idxf[:, 2:3], in1=e1[:],
                                               op0=OP.is_equal, op1=OP.add)
                pst = ps_t.tile([P, P], F32, tag="pt")
                nc.tensor.transpose(pst[:], S[:], ident[:])
                S_T = gt.tile([P, P], F32, tag="S_T")
                nc.vector.tensor_copy(S_T[:], pst[:])
                ppc = ps_a.tile([P, D], F32, tag="pa")
                nc.tensor.matmul(ppc[:], lhsT=S_T[:], rhs=E[:, b * D:(b + 1) * D],
                                 start=True, stop=True)
                diff = scr.tile([P, D], F32, tag="diff")
                nc.vector.scalar_tensor_tensor(diff[:], in0=ppc[:],
                                               scalar=-1.0 / 3.0,
                                               in1=E[:, b * D:(b + 1) * D],
                                               op0=OP.mult, op1=OP.add)
                sq2 = scr.tile([P, D], F32, tag="sq")
                nc.scalar.activation(sq2[:], diff[:], AF.Square,
                                     accum_out=dpq[:, b:b + 1])

            # ---- phase 5: centers prep (b2, biases, -2*centers^T) ----
            nc.scalar.activation(csq[:], centers_all[:], AF.Square,
                                 accum_out=b2col[:])
            pb2 = ps_t.tile([1, NSLOT], F32, tag="pt")
            nc.tensor.transpose(pb2[:], b2col[:], ident[0:NSLOT, 0:NSLOT])
            for b in range(BPC):
                nc.vector.tensor_tensor(comb[0:1, b * NSLOT:(b + 1) * NSLOT],
                                        pb2[:], nb_t[0:1, b * NSLOT:(b + 1) * NSLOT],
                                        op=OP.add)
            for k in range(KCH):
                ptc = ps_t.tile([P, NSLOT], F32, tag="pt")
                nc.tensor.transpose(ptc[:], centers_all[:, k * P:(k + 1) * P],
                                    ident[0:NSLOT, 0:NSLOT])
                nc.vector.tensor_scalar_mul(ct2[:, k * NSLOT:(k + 1) * NSLOT],
                                            ptc[:], -2.0)

            # ---- phase 6: negative scores + min ----
            for b in range(BPC):
                pss = ps_a.tile([P, NSLOT], F32, tag="pa")
                for k in range(KCH):
                    nc.tensor.matmul(pss[:],
                                     lhsT=ET[:, b * D + k * P:b * D + (k + 1) * P],
                                     rhs=ct2[:, k * NSLOT:(k + 1) * NSLOT],
                                     start=(k == 0), stop=False)
                nc.tensor.matmul(pss[:], lhsT=ones[:],
                                 rhs=comb[0:1, b * NSLOT:(b + 1) * NSLOT],
                                 start=False, stop=True)
                nc.vector.tensor_reduce(msc[:, b:b + 1], pss[:], axis=AX.X, op=OP.min)
                nc.vector.tensor_tensor(dnq[:, b:b + 1], msc[:, b:b + 1],
                                        a2[:, b:b + 1], op=OP.add)

            # clamp both squared distances at EPS (safe_sqrt)
            nc.vector.tensor_scalar_max(dsqs[:, 0:BPC], dpq[:], EPS)
            nc.vector.tensor_scalar_max(dsqs[:, BPC:2 * BPC], dnq[:], EPS)

            # ---- phase 7: loss ----
            ds = spool.tile([P, 2 * BPC], F32, tag="ds")
            nc.scalar.activation(ds[:], dsqs[:], AF.Sqrt)
            xs = spool.tile([P, BPC], F32, tag="xs")
            nc.vector.tensor_sub(xs[:], ds[:, 0:BPC], ds[:, BPC:2 * BPC])
            ex = spool.tile([P, BPC], F32, tag="ex")
            nc.scalar.activation(ex[:], xs[:], AF.Exp)
            ex1 = spool.tile([P, BPC], F32, tag="ex1")
            nc.vector.tensor_scalar_add(ex1[:], ex[:], 1.0)
            lp = spool.tile([P, BPC], F32, tag="lp")
            nc.scalar.activation(lp[:], ex1[:], AF.Ln)
            wl = spool.tile([P, BPC], F32, tag="wl")
            nc.vector.tensor_mul(wl[:], lp[:], lw_t[:])
            accrow = spool.tile([P, 1], F32, tag="accrow")
            nc.vector.tensor_reduce(accrow[:], wl[:], axis=AX.X, op=OP.add)
            total = spool.tile([1, 1], F32, tag="total")
            nc.gpsimd.tensor_reduce(total[:], accrow[:], axis=AX.C, op=OP.add)
            nc.sync.dma_start(out=out_h[:], in_=total[:])

    nc.finalize()
    return nc


def _get_nc():
    global _CACHED_NC
    if _CACHED_NC is None:
        _CACHED_NC = _build_nc()
    return _CACHED_NC


def _prep_inputs(embeddings, targets):
    """Host-side sharding: class-sorted, padded to 128-row class blocks."""
    emb = np.ascontiguousarray(np.asarray(embeddings, dtype=np.float32))
    tgt = np.asarray(targets).astype(np.int64)
    counts = np.bincount(tgt, minlength=C)
    if counts.max() > P:
        raise ValueError(f"class count {counts.max()} exceeds block size {P}")
    order = np.argsort(tgt, kind="stable")
    offs = np.zeros(C + 1, dtype=np.int64)
    np.cumsum(counts, out=offs[1:])

    emb_pad = np.zeros((NCORES, BPC * P, D), dtype=np.float32)
    rwm = np.zeros((NCORES, P, BPC * BPC), dtype=np.float32)
    lw = np.zeros((NCORES, P, BPC), dtype=np.float32)
    padbias = np.zeros((NCORES, 1, BPC * P), dtype=np.float32)
    negbias = np.zeros((NCORES, 1, BPC * NSLOT), dtype=np.float32)

    for slot in range(NSLOT):
        core, b = slot // BPC, slot % BPC
        if slot < C:
            cnt = int(counts[slot])
            rows = order[offs[slot]:offs[slot] + cnt]
            emb_pad[core, b * P:b * P + cnt] = emb[rows]
        else:
            cnt = 0
        padbias[core, 0, b * P + cnt:(b + 1) * P] = BIG
        if cnt:
            rwm[core, :cnt, b * BPC + b] = 1.0 / cnt
            if cnt >= 2:
                lw[core, :cnt, b] = 1.0
        # exclude own class and empty/pad class slots from the negative min
        nb = negbias[core, 0, b * NSLOT:(b + 1) * NSLOT]
        for j in range(NSLOT):
            if j == slot or j >= C or counts[j] == 0:
                nb[j] = BIG

    denom = float(counts[counts >= 2].sum())
    return emb_pad, rwm, lw, padbias, negbias, denom


def kernel(embeddings, targets, num_classes):
    emb_pad, rwm, lw, padbias, negbias, denom = _prep_inputs(embeddings, targets)
    nc = _get_nc()
    in_maps = [
        {
            "emb": emb_pad[i],
            "rwm": rwm[i],
            "lw": lw[i],
            "padbias": padbias[i],
            "negbias": negbias[i],
        }
        for i in range(NCORES)
    ]
    res = run_bass_kernel_spmd(nc, in_maps, core_ids=list(range(NCORES)))
    parts = [float(res.results[i]["out"][0, 0]) for i in range(NCORES)]
    loss = np.float32(np.sum(np.asarray(parts, dtype=np.float64)) / max(denom, 1.0))
    return np.asarray(loss, dtype=np.float32)


# revision 5
# speedup vs baseline: 1.2976x; 1.2976x over previous
"""AdaptiveTripletLoss distributed Trainium2 kernel (8 NeuronCores).

Strategy: shard by class. Host argsorts targets; each class becomes one
128-row padded block (max class count is ~105 for n=8192, C=100). 104
class slots = 13 blocks/core x 8 cores. Hardest-positive top-3 needs only
same-class distances, so each core computes 13 small 128x128 gram blocks
instead of a row-slab of the full 8192x8192 matrix. Class centers are
computed per-class locally and AllGathered. All floating-point loss math
runs on device; the host does data movement (sharding permutation) and the
final 8-way partial sum / count division.

Matmuls run in bf16 (selection ordering and center sums tolerate it; the
d_pos/d_neg value paths keep fp32 accumulation in PSUM). Top-3 one-hot is
built exactly with max8 + match_replace (no index arithmetic).
"""

import numpy as np
from concourse import bacc, mybir, tile, masks
from concourse.bass_types import AP
from concourse.bass_utils import run_bass_kernel_spmd

# Problem constants (hardcoded per harness contract)
N = 8192
D = 512
C = 100
NCORES = 8
BPC = 13              # class blocks per core
NSLOT = BPC * NCORES  # 104 class slots
P = 128               # rows per class block
KCH = D // P          # 4 contraction chunks
BIG = 1.0e4
EPS = 1.0e-12
SENT = 1.0e9          # match_replace sentinel (never present in negG)
REPL = 5.0            # match_replace imm (real negG values are <= ~1)
F32 = mybir.dt.float32
BF16 = mybir.dt.bfloat16

_CACHED_NC = None


def _build_nc():
    nc = bacc.Bacc("TRN2", target_bir_lowering=False, debug=False,
                   num_devices=NCORES)
    emb_h = nc.declare_dram_parameter("emb", [BPC * P, D], F32, isOutput=False)
    rw_h = nc.declare_dram_parameter("rwm", [P, BPC * BPC], BF16, isOutput=False)
    lw_h = nc.declare_dram_parameter("lw", [P, BPC], F32, isOutput=False)
    pb_h = nc.declare_dram_parameter("padbias", [1, BPC * P], F32, isOutput=False)
    nb_h = nc.declare_dram_parameter("negbias", [1, BPC * NSLOT], F32, isOutput=False)
    ic_h = nc.declare_dram_parameter("invc", [BPC, 1], F32, isOutput=False)
    out_h = nc.declare_dram_parameter("out", [1, 1], F32, isOutput=True)

    AX = mybir.AxisListType
    OP = mybir.AluOpType
    AF = mybir.ActivationFunctionType

    with tile.TileContext(nc) as tc:
        with (
            tc.tile_pool(name="const", bufs=1) as cpool,
            tc.tile_pool(name="big", bufs=1) as bpool,
            tc.tile_pool(name="sm", bufs=1) as spool,
            tc.tile_pool(name="scr", bufs=3) as scr,
            tc.tile_pool(name="gt", bufs=3) as gt,
            tc.tile_pool(name="st", bufs=3) as st,
            tc.tile_pool(name="ps_t", bufs=2, space="PSUM") as ps_t,
            tc.tile_pool(name="ps_a", bufs=3, space="PSUM") as ps_a,
            tc.tile_pool(name="ps_c", bufs=1, space="PSUM") as ps_c,
            tc.tile_pool(name="dram", bufs=1, space="DRAM") as dram,
        ):
            # ---- constants ----
            ident = cpool.tile([P, P], F32, tag="ident")
            masks.make_identity(nc, ident[:])
            ident_bf = cpool.tile([P, P], BF16, tag="ident_bf")
            masks.make_identity(nc, ident_bf[:])
            ones = cpool.tile([1, P], F32, tag="ones")
            nc.vector.memset(ones[:], 1.0)

            # ---- persistent tiles ----
            Eraw = bpool.tile([P, BPC * D], F32, tag="Eraw")
            E = bpool.tile([P, BPC * D], F32, tag="E")
            Eb = bpool.tile([P, BPC * D], BF16, tag="Eb")
            ETb = bpool.tile([P, BPC * D], BF16, tag="ETb")
            rw_t = spool.tile([P, BPC * BPC], BF16, tag="rw")
            lw_t = spool.tile([P, BPC], F32, tag="lwt")
            pb_t = spool.tile([1, BPC * P], F32, tag="pbt")
            nb_t = spool.tile([1, BPC * NSLOT], F32, tag="nbt")
            ic_t = spool.tile([BPC, 1], F32, tag="ict")
            ssq = spool.tile([P, BPC], F32, tag="ssq")
            nrm = spool.tile([P, BPC], F32, tag="nrm")
            rcp = spool.tile([P, BPC], F32, tag="rcp")
            tsc = spool.tile([P, BPC], F32, tag="tsc")
            a2 = spool.tile([P, BPC], F32, tag="a2")
            dpq = spool.tile([P, BPC], F32, tag="dpq")
            msc = spool.tile([P, BPC], F32, tag="msc")
            dnq = spool.tile([P, BPC], F32, tag="dnq")
            dsqs = spool.tile([P, 2 * BPC], F32, tag="dsqs")
            centers_l = spool.tile([BPC, D], F32, tag="centers_l")
            centers_all = spool.tile([NSLOT, D], F32, tag="centers_all")
            centers_bf = spool.tile([NSLOT, D], BF16, tag="centers_bf")
            csq = spool.tile([NSLOT, D], F32, tag="csq")
            b2col = spool.tile([NSLOT, 1], F32, tag="b2col")
            comb = spool.tile([1, BPC * NSLOT], F32, tag="comb")
            ct2 = spool.tile([P, KCH * NSLOT], BF16, tag="ct2")

            # ---- input DMAs ----
            for b in range(BPC):
                nc.sync.dma_start(out=Eraw[:, b * D:(b + 1) * D],
                                  in_=emb_h[b * P:(b + 1) * P, :])
            nc.sync.dma_start(out=rw_t[:], in_=rw_h[:])
            nc.sync.dma_start(out=lw_t[:], in_=lw_h[:])
            nc.sync.dma_start(out=pb_t[:], in_=pb_h[:])
            nc.sync.dma_start(out=nb_t[:], in_=nb_h[:])
            nc.sync.dma_start(out=ic_t[:], in_=ic_h[:])

            # ---- phase 1: normalize (per block, engines alternating) ----
            for b in range(BPC):
                bsl = slice(b * D, (b + 1) * D)
                sq = scr.tile([P, D], F32, tag="sq")
                if b % 2 == 0:
                    nc.vector.scalar_tensor_tensor(
                        sq[:], in0=Eraw[:, bsl], scalar=1.0, in1=Eraw[:, bsl],
                        op0=OP.mult, op1=OP.mult, accum_out=ssq[:, b:b + 1])
                else:
                    nc.scalar.activation(sq[:], Eraw[:, bsl], AF.Square,
                                         accum_out=ssq[:, b:b + 1])
            nc.scalar.activation(nrm[:], ssq[:], AF.Sqrt)
            nc.vector.tensor_scalar_max(nrm[:], nrm[:], EPS)
            nc.vector.reciprocal(rcp[:], nrm[:])
            nc.vector.tensor_mul(tsc[:], rcp[:], rcp[:])
            nc.vector.tensor_mul(a2[:], ssq[:], tsc[:])
            for b in range(BPC):
                bsl = slice(b * D, (b + 1) * D)
                nc.scalar.activation(E[:, bsl], Eraw[:, bsl], AF.Copy,
                                     scale=rcp[:, b:b + 1])
                nc.gpsimd.tensor_copy(Eb[:, bsl], E[:, bsl])

            # ---- phase 2: per-class centers + AllGather (critical path) ----
            pcn = ps_c.tile([BPC, D], F32, tag="pcn")
            for b in range(BPC):
                nc.tensor.matmul(pcn[:], lhsT=rw_t[:, b * BPC:(b + 1) * BPC],
                                 rhs=Eb[:, b * D:(b + 1) * D],
                                 start=(b == 0), stop=(b == BPC - 1))
            nc.vector.tensor_scalar_mul(centers_l[:], pcn[:], ic_t[:])
            cc_in = dram.tile([BPC, D], F32, tag="cc_in")
            cc_out = dram.tile([NSLOT, D], F32, addr_space="Shared", tag="cc_out")
            nc.sync.dma_start(out=cc_in[:], in_=centers_l[:])
            nc.gpsimd.collective_compute(
                "AllGather", OP.bypass,
                replica_groups=[list(range(NCORES))],
                ins=[cc_in[:].opt()],
                outs=[cc_out[:].opt()],
            )
            nc.sync.dma_start(out=centers_all[:], in_=cc_out[:])

            # ---- phase 3: transpose E blocks (bf16, batched copy-out) ----
            for b in range(BPC):
                pt = ps_t.tile([P, D], BF16, tag="pt")
                for k in range(KCH):
                    nc.tensor.transpose(pt[:, k * P:(k + 1) * P],
                                        Eb[:, b * D + k * P:b * D + (k + 1) * P],
                                        ident_bf[:])
                nc.vector.tensor_copy(ETb[:, b * D:(b + 1) * D], pt[:])

            # ---- phase 4: gram + exact top3 + pos-center + d_pos^2 ----
            for b in range(BPC):
                pg = ps_a.tile([P, P], F32, tag="pa")
                for k in range(KCH):
                    sl = slice(b * D + k * P, b * D + (k + 1) * P)
                    nc.tensor.matmul(pg[:], lhsT=ETb[:, sl], rhs=ETb[:, sl],
                                     start=(k == 0), stop=False)
                nc.tensor.matmul(pg[:], lhsT=ones[:],
                                 rhs=pb_t[0:1, b * P:(b + 1) * P],
                                 start=False, stop=True)
                negG = gt.tile([P, P], F32, tag="negG")
                nc.vector.tensor_scalar_mul(negG[:], pg[:], -1.0)
                v8 = st.tile([P, 8], F32, tag="v8")
                nc.vector.max(v8[:], negG[:])
                mr8 = st.tile([P, 8], F32, tag="mr8")
                nc.vector.tensor_copy(mr8[:, 0:3], v8[:, 0:3])
                nc.vector.memset(mr8[:, 3:8], SENT)
                Gm = gt.tile([P, P], F32, tag="Gm")
                nc.vector.match_replace(Gm[:], mr8[:], negG[:], REPL)
                Sb = gt.tile([P, P], BF16, tag="Sb")
                nc.vector.tensor_scalar(Sb[:], Gm[:], REPL - 1.0, None, op0=OP.is_ge)
                pst = ps_t.tile([P, P], BF16, tag="pt")
                nc.tensor.transpose(pst[:], Sb[:], ident_bf[:])
                S_T = gt.tile([P, P], BF16, tag="S_T")
                nc.vector.tensor_copy(S_T[:], pst[:])
                ppc = ps_a.tile([P, D], F32, tag="pa")
                nc.tensor.matmul(ppc[:], lhsT=S_T[:], rhs=Eb[:, b * D:(b + 1) * D],
                                 start=True, stop=True)
                diff = scr.tile([P, D], F32, tag="diff")
                nc.vector.scalar_tensor_tensor(diff[:], in0=ppc[:],
                                               scalar=-1.0 / 3.0,
                                               in1=E[:, b * D:(b + 1) * D],
                                               op0=OP.mult, op1=OP.add)
                sq2 = scr.tile([P, D], F32, tag="sq")
                if b % 2 == 0:
                    nc.scalar.activation(sq2[:], diff[:], AF.Square,
                                         accum_out=dpq[:, b:b + 1])
                else:
                    nc.vector.scalar_tensor_tensor(
                        sq2[:], in0=diff[:], scalar=1.0, in1=diff[:],
                        op0=OP.mult, op1=OP.mult, accum_out=dpq[:, b:b + 1])

            # ---- phase 5: centers prep ----
            nc.scalar.activation(csq[:], centers_all[:], AF.Square,
                                 accum_out=b2col[:])
            pb2 = ps_t.tile([1, NSLOT], F32, tag="pt")
            nc.tensor.transpose(pb2[:], b2col[:], ident[0:NSLOT, 0:NSLOT])
            # comb[0, b*104 + j] = b2[j] + negbias[b*104 + j]  (one stride-0 op)
            pb2_ap = pb2[:]
            pb2_b = AP(pb2_ap.tensor, pb2_ap.offset,
                       [pb2_ap.ap[0], [0, BPC], [1, NSLOT]])
            comb_ap = comb[:]
            comb_3 = AP(comb_ap.tensor, comb_ap.offset,
                        [comb_ap.ap[0], [NSLOT, BPC], [1, NSLOT]])
            nb_ap = nb_t[:]
            nb_3 = AP(nb_ap.tensor, nb_ap.offset,
                      [nb_ap.ap[0], [NSLOT, BPC], [1, NSLOT]])
            nc.vector.tensor_tensor(comb_3, pb2_b, nb_3, op=OP.add)
            nc.gpsimd.tensor_copy(centers_bf[:], centers_all[:])
            ptc = ps_t.tile([P, KCH * NSLOT], BF16, tag="pt")
            for k in range(KCH):
                nc.tensor.transpose(ptc[:, k * NSLOT:(k + 1) * NSLOT],
                                    centers_bf[:, k * P:(k + 1) * P],
                                    ident_bf[0:NSLOT, 0:NSLOT])
            nc.vector.tensor_scalar_mul(ct2[:], ptc[:], -2.0)

            # ---- phase 6: negative scores + min ----
            for b in range(BPC):
                pss = ps_a.tile([P, NSLOT], F32, tag="pa")
                for k in range(KCH):
                    nc.tensor.matmul(pss[:],
                                     lhsT=ETb[:, b * D + k * P:b * D + (k + 1) * P],
                                     rhs=ct2[:, k * NSLOT:(k + 1) * NSLOT],
                                     start=(k == 0), stop=False)
                nc.tensor.matmul(pss[:], lhsT=ones[:],
                                 rhs=comb[0:1, b * NSLOT:(b + 1) * NSLOT],
                                 start=False, stop=True)
                nc.vector.tensor_reduce(msc[:, b:b + 1], pss[:], axis=AX.X,
                                        op=OP.min)

            # ---- phase 7: loss tail ----
            nc.vector.tensor_tensor(dnq[:], msc[:], a2[:], op=OP.add)
            nc.vector.tensor_scalar_max(dsqs[:, 0:BPC], dpq[:], EPS)
            nc.vector.tensor_scalar_max(dsqs[:, BPC:2 * BPC], dnq[:], EPS)
            ds = spool.tile([P, 2 * BPC], F32, tag="ds")
            nc.scalar.activation(ds[:], dsqs[:], AF.Sqrt)
            xs = spool.tile([P, BPC], F32, tag="xs")
            nc.vector.tensor_sub(xs[:], ds[:, 0:BPC], ds[:, BPC:2 * BPC])
            ex = spool.tile([P, BPC], F32, tag="ex")
            nc.scalar.activation(ex[:], xs[:], AF.Exp)
            ex1 = spool.tile([P, BPC], F32, tag="ex1")
            nc.vector.tensor_scalar_add(ex1[:], ex[:], 1.0)
            lp = spool.tile([P, BPC], F32, tag="lp")
            nc.scalar.activation(lp[:], ex1[:], AF.Ln)
            wl = spool.tile([P, BPC], F32, tag="wl")
            nc.vector.tensor_mul(wl[:], lp[:], lw_t[:])
            accrow = spool.tile([P, 1], F32, tag="accrow")
            nc.vector.tensor_reduce(accrow[:], wl[:], axis=AX.X, op=OP.add)
            total = spool.tile([1, 1], F32, tag="total")
            nc.gpsimd.tensor_reduce(total[:], accrow[:], axis=AX.C, op=OP.add)
            nc.sync.dma_start(out=out_h[:], in_=total[:])

    nc.finalize()
    return nc


def _get_nc():
    global _CACHED_NC
    if _CACHED_NC is None:
        _CACHED_NC = _build_nc()
    return _CACHED_NC


def _prep_inputs(embeddings, targets):
    """Host-side sharding: class-sorted, padded to 128-row class blocks."""
    import ml_dtypes
    emb = np.ascontiguousarray(np.asarray(embeddings, dtype=np.float32))
    tgt = np.asarray(targets).astype(np.int64)
    counts = np.bincount(tgt, minlength=C)
    if counts.max() > P:
        raise ValueError(f"class count {counts.max()} exceeds block size {P}")
    order = np.argsort(tgt, kind="stable")
    offs = np.zeros(C + 1, dtype=np.int64)
    np.cumsum(counts, out=offs[1:])

    emb_pad = np.zeros((NCORES, BPC * P, D), dtype=np.float32)
    rwm = np.zeros((NCORES, P, BPC * BPC), dtype=ml_dtypes.bfloat16)
    lw = np.zeros((NCORES, P, BPC), dtype=np.float32)
    padbias = np.zeros((NCORES, 1, BPC * P), dtype=np.float32)
    negbias = np.zeros((NCORES, 1, BPC * NSLOT), dtype=np.float32)
    invc = np.zeros((NCORES, BPC, 1), dtype=np.float32)

    for slot in range(NSLOT):
        core, b = slot // BPC, slot % BPC
        if slot < C:
            cnt = int(counts[slot])
            rows = order[offs[slot]:offs[slot] + cnt]
            emb_pad[core, b * P:b * P + cnt] = emb[rows]
        else:
            cnt = 0
        padbias[core, 0, b * P + cnt:(b + 1) * P] = BIG
        if cnt:
            rwm[core, :cnt, b * BPC + b] = 1.0
            invc[core, b, 0] = 1.0 / cnt
            if cnt >= 2:
                lw[core, :cnt, b] = 1.0
        # exclude own class and empty/pad class slots from the negative min
        nb = negbias[core, 0, b * NSLOT:(b + 1) * NSLOT]
        for j in range(NSLOT):
            if j == slot or j >= C or counts[j] == 0:
                nb[j] = BIG

    denom = float(counts[counts >= 2].sum())
    return emb_pad, rwm, lw, padbias, negbias, invc, denom


def _make_in_maps(emb_pad, rwm, lw, padbias, negbias, invc):
    return [
        {
            "emb": emb_pad[i],
            "rwm": rwm[i],
            "lw": lw[i],
            "padbias": padbias[i],
            "negbias": negbias[i],
            "invc": invc[i],
        }
        for i in range(NCORES)
    ]


def kernel(embeddings, targets, num_classes):
    emb_pad, rwm, lw, padbias, negbias, invc, denom = _prep_inputs(
        embeddings, targets)
    nc = _get_nc()
    in_maps = _make_in_maps(emb_pad, rwm, lw, padbias, negbias, invc)
    res = run_bass_kernel_spmd(nc, in_maps, core_ids=list(range(NCORES)))
    parts = [float(res.results[i]["out"][0, 0]) for i in range(NCORES)]
    loss = np.float32(np.sum(np.asarray(parts, dtype=np.float64)) / max(denom, 1.0))
    return np.asarray(loss, dtype=np.float32)


# revision 6
# speedup vs baseline: 1.4699x; 1.1328x over previous
"""AdaptiveTripletLoss distributed Trainium2 kernel (8 NeuronCores).

Strategy: shard by class. Host argsorts targets; each class becomes one
128-row padded block (max class count is ~105 for n=8192, C=100). 104
class slots = 13 blocks/core x 8 cores. Hardest-positive top-3 needs only
same-class distances, so each core computes 13 small 128x128 gram blocks
instead of a row-slab of the full 8192x8192 matrix. Class centers are
computed per-class locally and AllGathered. All floating-point loss math
runs on device; the host does data movement (sharding permutation) and the
final 8-way partial sum / count division.

Matmuls run in bf16 (selection ordering and center sums tolerate it; the
d_pos/d_neg value paths keep fp32 accumulation in PSUM). Top-3 one-hot is
built exactly with max8 + match_replace (no index arithmetic).
"""

import numpy as np
from concourse import bacc, mybir, tile, masks
from concourse.bass_types import AP
from concourse.bass_utils import run_bass_kernel_spmd

# Problem constants (hardcoded per harness contract)
N = 8192
D = 512
C = 100
NCORES = 8
BPC = 13              # class blocks per core
NSLOT = BPC * NCORES  # 104 class slots
P = 128               # rows per class block
KCH = D // P          # 4 contraction chunks
BIG = 1.0e4
EPS = 1.0e-12
SENT = 1.0e9          # match_replace sentinel (never present in negG)
REPL = 5.0            # match_replace imm (real negG values are <= ~1)
F32 = mybir.dt.float32
BF16 = mybir.dt.bfloat16

_CACHED_NC = None


def _build_nc():
    nc = bacc.Bacc("TRN2", target_bir_lowering=False, debug=False,
                   num_devices=NCORES)
    emb_h = nc.declare_dram_parameter("emb", [BPC * P, D], F32, isOutput=False)
    rw_h = nc.declare_dram_parameter("rwm", [P, BPC * BPC], BF16, isOutput=False)
    lw_h = nc.declare_dram_parameter("lw", [P, BPC], F32, isOutput=False)
    pb_h = nc.declare_dram_parameter("padbias", [1, BPC * P], F32, isOutput=False)
    nb_h = nc.declare_dram_parameter("negbias", [1, BPC * NSLOT], F32, isOutput=False)
    ic_h = nc.declare_dram_parameter("invc", [BPC, 1], F32, isOutput=False)
    out_h = nc.declare_dram_parameter("out", [1, 1], F32, isOutput=True)

    AX = mybir.AxisListType
    OP = mybir.AluOpType
    AF = mybir.ActivationFunctionType

    with tile.TileContext(nc) as tc:
        with (
            tc.tile_pool(name="const", bufs=1) as cpool,
            tc.tile_pool(name="big", bufs=1) as bpool,
            tc.tile_pool(name="sm", bufs=1) as spool,
            tc.tile_pool(name="scr", bufs=3) as scr,
            tc.tile_pool(name="gt", bufs=3) as gt,
            tc.tile_pool(name="st", bufs=3) as st,
            tc.tile_pool(name="ps_t", bufs=2, space="PSUM") as ps_t,
            tc.tile_pool(name="ps_a", bufs=3, space="PSUM") as ps_a,
            tc.tile_pool(name="ps_c", bufs=1, space="PSUM") as ps_c,
            tc.tile_pool(name="dram", bufs=1, space="DRAM") as dram,
        ):
            # ---- constants ----
            ident = cpool.tile([P, P], F32, tag="ident")
            masks.make_identity(nc, ident[:])
            ident_bf = cpool.tile([P, P], BF16, tag="ident_bf")
            masks.make_identity(nc, ident_bf[:])
            ones = cpool.tile([1, P], F32, tag="ones")
            nc.vector.memset(ones[:], 1.0)

            # ---- persistent tiles ----
            Eraw = bpool.tile([P, BPC * D], F32, tag="Eraw")
            Eb = bpool.tile([P, BPC * D], BF16, tag="Eb")
            ETb = bpool.tile([P, BPC * D], BF16, tag="ETb")
            rw_t = spool.tile([P, BPC * BPC], BF16, tag="rw")
            lw_t = spool.tile([P, BPC], F32, tag="lwt")
            pb_t = spool.tile([1, BPC * P], F32, tag="pbt")
            nb_t = spool.tile([1, BPC * NSLOT], F32, tag="nbt")
            ic_t = spool.tile([BPC, 1], F32, tag="ict")
            ssq = spool.tile([P, BPC], F32, tag="ssq")
            nrm = spool.tile([P, BPC], F32, tag="nrm")
            rcp = spool.tile([P, BPC], F32, tag="rcp")
            tsc = spool.tile([P, BPC], F32, tag="tsc")
            a2 = spool.tile([P, BPC], F32, tag="a2")
            dpq = spool.tile([P, BPC], F32, tag="dpq")
            msc = spool.tile([P, BPC], F32, tag="msc")
            dnq = spool.tile([P, BPC], F32, tag="dnq")
            dsqs = spool.tile([P, 2 * BPC], F32, tag="dsqs")
            centers_l = spool.tile([BPC, D], F32, tag="centers_l")
            centers_all = spool.tile([NSLOT, D], F32, tag="centers_all")
            centers_bf = spool.tile([NSLOT, D], BF16, tag="centers_bf")
            csq = spool.tile([NSLOT, D], F32, tag="csq")
            b2col = spool.tile([NSLOT, 1], F32, tag="b2col")
            comb = spool.tile([1, BPC * NSLOT], F32, tag="comb")
            ct2 = spool.tile([P, KCH * NSLOT], BF16, tag="ct2")

            # ---- dummy collective: absorb comm-init barrier early ----
            dummy_sb = spool.tile([1, 1], F32, tag="dummy_sb")
            nc.vector.memset(dummy_sb[:], 0.0)
            dummy_in = dram.tile([1, 1], F32, tag="dummy_in")
            dummy_out = dram.tile([NCORES, 1], F32, addr_space="Shared",
                                  tag="dummy_out")
            nc.sync.dma_start(out=dummy_in[:], in_=dummy_sb[:])
            nc.gpsimd.collective_compute(
                "AllGather", OP.bypass,
                replica_groups=[list(range(NCORES))],
                ins=[dummy_in[:].opt()],
                outs=[dummy_out[:].opt()],
            )

            # ---- input DMAs ----
            for b in range(BPC):
                nc.sync.dma_start(out=Eraw[:, b * D:(b + 1) * D],
                                  in_=emb_h[b * P:(b + 1) * P, :])
            nc.sync.dma_start(out=rw_t[:], in_=rw_h[:])
            nc.sync.dma_start(out=lw_t[:], in_=lw_h[:])
            nc.sync.dma_start(out=pb_t[:], in_=pb_h[:])
            nc.sync.dma_start(out=nb_t[:], in_=nb_h[:])
            nc.sync.dma_start(out=ic_t[:], in_=ic_h[:])

            # ---- phase 1+2: normalize per block, then class-center matmul ----
            pcn = ps_c.tile([BPC, D], F32, tag="pcn")
            for b in range(BPC):
                bsl = slice(b * D, (b + 1) * D)
                sq = scr.tile([P, D], F32, tag="sq")
                if b % 2 == 0:
                    nc.vector.scalar_tensor_tensor(
                        sq[:], in0=Eraw[:, bsl], scalar=1.0, in1=Eraw[:, bsl],
                        op0=OP.mult, op1=OP.mult, accum_out=ssq[:, b:b + 1])
                else:
                    nc.scalar.activation(sq[:], Eraw[:, bsl], AF.Square,
                                         accum_out=ssq[:, b:b + 1])
                nc.scalar.activation(nrm[:, b:b + 1], ssq[:, b:b + 1], AF.Sqrt)
                nc.vector.tensor_scalar_max(nrm[:, b:b + 1], nrm[:, b:b + 1], EPS)
                nc.vector.reciprocal(rcp[:, b:b + 1], nrm[:, b:b + 1])
                nc.scalar.activation(Eb[:, bsl], Eraw[:, bsl], AF.Copy,
                                     scale=rcp[:, b:b + 1])
                nc.tensor.matmul(pcn[:], lhsT=rw_t[:, b * BPC:(b + 1) * BPC],
                                 rhs=Eb[:, bsl],
                                 start=(b == 0), stop=(b == BPC - 1))
            nc.vector.tensor_scalar_mul(centers_l[:], pcn[:], ic_t[:])
            nc.vector.tensor_mul(tsc[:], rcp[:], rcp[:])
            nc.vector.tensor_mul(a2[:], ssq[:], tsc[:])
            cc_in = dram.tile([BPC, D], F32, tag="cc_in")
            cc_out = dram.tile([NSLOT, D], F32, addr_space="Shared", tag="cc_out")
            nc.sync.dma_start(out=cc_in[:], in_=centers_l[:])
            nc.gpsimd.collective_compute(
                "AllGather", OP.bypass,
                replica_groups=[list(range(NCORES))],
                ins=[cc_in[:].opt()],
                outs=[cc_out[:].opt()],
            )
            nc.sync.dma_start(out=centers_all[:], in_=cc_out[:])

            # ---- phase 3: transpose E blocks (bf16, batched copy-out) ----
            for b in range(BPC):
                pt = ps_t.tile([P, D], BF16, tag="pt")
                for k in range(KCH):
                    nc.tensor.transpose(pt[:, k * P:(k + 1) * P],
                                        Eb[:, b * D + k * P:b * D + (k + 1) * P],
                                        ident_bf[:])
                if b % 2 == 0:
                    nc.vector.tensor_copy(ETb[:, b * D:(b + 1) * D], pt[:])
                else:
                    nc.scalar.activation(ETb[:, b * D:(b + 1) * D], pt[:], AF.Copy)

            # ---- phase 4: gram + exact top3 + pos-center + d_pos^2 ----
            for b in range(BPC):
                pg = ps_a.tile([P, P], F32, tag="pa")
                for k in range(KCH):
                    sl = slice(b * D + k * P, b * D + (k + 1) * P)
                    nc.tensor.matmul(pg[:], lhsT=ETb[:, sl], rhs=ETb[:, sl],
                                     start=(k == 0), stop=False)
                nc.tensor.matmul(pg[:], lhsT=ones[:],
                                 rhs=pb_t[0:1, b * P:(b + 1) * P],
                                 start=False, stop=True)
                negG = gt.tile([P, P], F32, tag="negG")
                nc.vector.tensor_scalar_mul(negG[:], pg[:], -1.0)
                v8 = st.tile([P, 8], F32, tag="v8")
                nc.vector.max(v8[:], negG[:])
                nc.vector.memset(v8[:, 3:8], SENT)
                Gm = gt.tile([P, P], F32, tag="Gm")
                nc.vector.match_replace(Gm[:], v8[:], negG[:], REPL)
                Sb = gt.tile([P, P], BF16, tag="Sb")
                nc.vector.tensor_scalar(Sb[:], Gm[:], REPL - 1.0, None, op0=OP.is_ge)
                pst = ps_t.tile([P, P], BF16, tag="pt")
                nc.tensor.transpose(pst[:], Sb[:], ident_bf[:])
                S_T = gt.tile([P, P], BF16, tag="S_T")
                nc.vector.tensor_copy(S_T[:], pst[:])
                ppc = ps_a.tile([P, D], F32, tag="pa")
                nc.tensor.matmul(ppc[:], lhsT=S_T[:], rhs=Eb[:, b * D:(b + 1) * D],
                                 start=True, stop=True)
                diff = scr.tile([P, D], F32, tag="diff")
                nc.vector.scalar_tensor_tensor(diff[:], in0=ppc[:],
                                               scalar=-1.0 / 3.0,
                                               in1=Eb[:, b * D:(b + 1) * D],
                                               op0=OP.mult, op1=OP.add)
                sq2 = scr.tile([P, D], F32, tag="sq")
                if b % 2 == 0:
                    nc.scalar.activation(sq2[:], diff[:], AF.Square,
                                         accum_out=dpq[:, b:b + 1])
                else:
                    nc.vector.scalar_tensor_tensor(
                        sq2[:], in0=diff[:], scalar=1.0, in1=diff[:],
                        op0=OP.mult, op1=OP.mult, accum_out=dpq[:, b:b + 1])

            # ---- phase 5: centers prep ----
            nc.scalar.activation(csq[:], centers_all[:], AF.Square,
                                 accum_out=b2col[:])
            pb2 = ps_t.tile([1, NSLOT], F32, tag="pt")
            nc.tensor.transpose(pb2[:], b2col[:], ident[0:NSLOT, 0:NSLOT])
            # comb[0, b*104 + j] = b2[j] + negbias[b*104 + j]  (one stride-0 op)
            pb2_ap = pb2[:]
            pb2_b = AP(pb2_ap.tensor, pb2_ap.offset,
                       [pb2_ap.ap[0], [0, BPC], [1, NSLOT]])
            comb_ap = comb[:]
            comb_3 = AP(comb_ap.tensor, comb_ap.offset,
                        [comb_ap.ap[0], [NSLOT, BPC], [1, NSLOT]])
            nb_ap = nb_t[:]
            nb_3 = AP(nb_ap.tensor, nb_ap.offset,
                      [nb_ap.ap[0], [NSLOT, BPC], [1, NSLOT]])
            nc.vector.tensor_tensor(comb_3, pb2_b, nb_3, op=OP.add)
            nc.scalar.activation(centers_bf[:], centers_all[:], AF.Copy)
            ptc = ps_t.tile([P, KCH * NSLOT], BF16, tag="pt")
            for k in range(KCH):
                nc.tensor.transpose(ptc[:, k * NSLOT:(k + 1) * NSLOT],
                                    centers_bf[:, k * P:(k + 1) * P],
                                    ident_bf[0:NSLOT, 0:NSLOT])
            nc.vector.tensor_scalar_mul(ct2[:], ptc[:], -2.0)

            # ---- phase 6: negative scores + min ----
            for b in range(BPC):
                pss = ps_a.tile([P, NSLOT], F32, tag="pa")
                for k in range(KCH):
                    nc.tensor.matmul(pss[:],
                                     lhsT=ETb[:, b * D + k * P:b * D + (k + 1) * P],
                                     rhs=ct2[:, k * NSLOT:(k + 1) * NSLOT],
                                     start=(k == 0), stop=False)
                nc.tensor.matmul(pss[:], lhsT=ones[:],
                                 rhs=comb[0:1, b * NSLOT:(b + 1) * NSLOT],
                                 start=False, stop=True)
                nc.vector.tensor_reduce(msc[:, b:b + 1], pss[:], axis=AX.X,
                                        op=OP.min)

            # ---- phase 7: loss tail ----
            nc.vector.tensor_tensor(dnq[:], msc[:], a2[:], op=OP.add)
            nc.vector.tensor_scalar_max(dsqs[:, 0:BPC], dpq[:], EPS)
            nc.vector.tensor_scalar_max(dsqs[:, BPC:2 * BPC], dnq[:], EPS)
            ds = spool.tile([P, 2 * BPC], F32, tag="ds")
            nc.scalar.activation(ds[:], dsqs[:], AF.Sqrt)
            xs = spool.tile([P, BPC], F32, tag="xs")
            nc.vector.tensor_sub(xs[:], ds[:, 0:BPC], ds[:, BPC:2 * BPC])
            ex = spool.tile([P, BPC], F32, tag="ex")
            nc.scalar.activation(ex[:], xs[:], AF.Exp)
            ex1 = spool.tile([P, BPC], F32, tag="ex1")
            nc.vector.tensor_scalar_add(ex1[:], ex[:], 1.0)
            lp = spool.tile([P, BPC], F32, tag="lp")
            nc.scalar.activation(lp[:], ex1[:], AF.Ln)
            wl = spool.tile([P, BPC], F32, tag="wl")
            nc.vector.tensor_mul(wl[:], lp[:], lw_t[:])
            accrow = spool.tile([P, 1], F32, tag="accrow")
            nc.vector.tensor_reduce(accrow[:], wl[:], axis=AX.X, op=OP.add)
            total = spool.tile([1, 1], F32, tag="total")
            nc.gpsimd.tensor_reduce(total[:], accrow[:], axis=AX.C, op=OP.add)
            nc.sync.dma_start(out=out_h[:], in_=total[:])

    nc.finalize()
    return nc


def _get_nc():
    global _CACHED_NC
    if _CACHED_NC is None:
        _CACHED_NC = _build_nc()
    return _CACHED_NC


def _prep_inputs(embeddings, targets):
    """Host-side sharding: class-sorted, padded to 128-row class blocks."""
    import ml_dtypes
    emb = np.ascontiguousarray(np.asarray(embeddings, dtype=np.float32))
    tgt = np.asarray(targets).astype(np.int64)
    counts = np.bincount(tgt, minlength=C)
    if counts.max() > P:
        raise ValueError(f"class count {counts.max()} exceeds block size {P}")
    order = np.argsort(tgt, kind="stable")
    offs = np.zeros(C + 1, dtype=np.int64)
    np.cumsum(counts, out=offs[1:])

    emb_pad = np.zeros((NCORES, BPC * P, D), dtype=np.float32)
    rwm = np.zeros((NCORES, P, BPC * BPC), dtype=ml_dtypes.bfloat16)
    lw = np.zeros((NCORES, P, BPC), dtype=np.float32)
    padbias = np.zeros((NCORES, 1, BPC * P), dtype=np.float32)
    negbias = np.zeros((NCORES, 1, BPC * NSLOT), dtype=np.float32)
    invc = np.zeros((NCORES, BPC, 1), dtype=np.float32)

    for slot in range(NSLOT):
        core, b = slot // BPC, slot % BPC
        if slot < C:
            cnt = int(counts[slot])
            rows = order[offs[slot]:offs[slot] + cnt]
            emb_pad[core, b * P:b * P + cnt] = emb[rows]
        else:
            cnt = 0
        padbias[core, 0, b * P + cnt:(b + 1) * P] = BIG
        if cnt:
            rwm[core, :cnt, b * BPC + b] = 1.0
            invc[core, b, 0] = 1.0 / cnt
            if cnt >= 2:
                lw[core, :cnt, b] = 1.0
        # exclude own class and empty/pad class slots from the negative min
        nb = negbias[core, 0, b * NSLOT:(b + 1) * NSLOT]
        for j in range(NSLOT):
            if j == slot or j >= C or counts[j] == 0:
                nb[j] = BIG

    denom = float(counts[counts >= 2].sum())
    return emb_pad, rwm, lw, padbias, negbias, invc, denom


def _make_in_maps(emb_pad, rwm, lw, padbias, negbias, invc):
    return [
        {
            "emb": emb_pad[i],
            "rwm": rwm[i],
            "lw": lw[i],
            "padbias": padbias[i],
            "negbias": negbias[i],
            "invc": invc[i],
        }
        for i in range(NCORES)
    ]


def kernel(embeddings, targets, num_classes):
    emb_pad, rwm, lw, padbias, negbias, invc, denom = _prep_inputs(
        embeddings, targets)
    nc = _get_nc()
    in_maps = _make_in_maps(emb_pad, rwm, lw, padbias, negbias, invc)
    res = run_bass_kernel_spmd(nc, in_maps, core_ids=list(range(NCORES)))
    parts = [float(res.results[i]["out"][0, 0]) for i in range(NCORES)]
    loss = np.float32(np.sum(np.asarray(parts, dtype=np.float64)) / max(denom, 1.0))
    return np.asarray(loss, dtype=np.float32)
